# revision 1
# baseline (speedup 1.0000x reference)
"""Causal self-attention (B=4, T=2048, C=1024, H=16) on 8 trn2 NeuronCores.

Sharding: data-parallel over batch (4) x tensor-parallel over heads (2 groups
of 8).  Core c handles batch c//2, head group c%2.  Each core computes
qkv projection for its heads, causal flash-style attention, and a partial
output projection (over its 512 rows of w_proj).  The host sums the two
TP partials per batch and adds the bias.

Device layout notes:
  - host feeds x^T (feature-major) so the contraction dim (C) lands on SBUF
    partitions for the QKV matmuls with no on-device transpose.
  - Q^T,K^T produced feature-on-partition ([64h+d -> (p,sub)]), V produced
    token-on-partition with a ones column per head, so P@V and the softmax
    denominator come from a single [V|1] matmul per head (M=65; denominator
    in psum row 64).  The head pair shares one 2-bank psum tile.
  - S^T tiles ([t2,t1]) are computed per (head-pair, q-chunk) with the two
    heads row-tiled (K=64 each, array rows 0-63 / 64-127); softmax is
    exp-without-max (scores are ~N(0,1); max over 268M scores ~ 6.5, safe
    in fp32), masked additively only on the 128-wide diagonal slab; fully
    masked columns are simply never computed/streamed.
  - normalization: one DVE copy frees the psum bank, DVE reciprocal of the
    denominator row, a DRAM-roundtrip broadcast DMA replicates it across 64
    partitions (stride-0 DRAM source APs are legal, SBUF ones are not),
    DVE multiplies into O^T, and odd heads take a partition-shifting
    SBUF->SBUF DMA into the upper half of O^T.
  - output projection consumes O^T directly as lhsT (contraction = head dim
    on partitions); host pre-permutes w_proj rows to match the O^T layout.
"""

import sys

sys.path.insert(0, "/opt/trn_rl_repo")

import numpy as np

import concourse.bass as bass
import concourse.bacc as bacc
import concourse.mybir as mybir
import concourse.tile as tile
from concourse.bass_utils import run_bass_kernel_spmd

F32 = mybir.dt.float32
P = 128
B, T, C = 4, 2048, 1024
H, D = 16, 64
NCORES = 8
TP = 2               # head-parallel groups
HL = H // TP         # 8 heads per core
CW = HL * D          # 512 head-cols per core
KS = C // P          # 8 contraction subtiles
NT = T // P          # 16 token tiles
MASK_NEG = -30000.0
SCALE = float(1.0 / np.sqrt(D))

_CACHE = {}


def _build_module():
    nc = bacc.Bacc("TRN2", target_bir_lowering=False, debug=False,
                   num_devices=NCORES)
    xT = nc.dram_tensor("xT", (P, KS, T), F32, kind="ExternalInput").ap()
    wqk = nc.dram_tensor("wqk", (8, P, KS, P), F32, kind="ExternalInput").ap()
    wv = nc.dram_tensor("wv", (P, KS, CW), F32, kind="ExternalInput").ap()
    wp = nc.dram_tensor("wp", (P, 4, C), F32, kind="ExternalInput").ap()
    msk = nc.dram_tensor("msk", (P, P), F32, kind="ExternalInput").ap()
    y = nc.dram_tensor("y", (NT, P, C), F32, kind="ExternalOutput").ap()

    Exp = mybir.ActivationFunctionType.Exp
    Add = mybir.AluOpType.add

    with tile.TileContext(nc) as tc, \
         tc.tile_pool(name="per", bufs=1) as per, \
         tc.tile_pool(name="strm", bufs=2) as strm, \
         tc.tile_pool(name="pp", bufs=2) as pp, \
         tc.tile_pool(name="pss", bufs=2, space="PSUM") as pss, \
         tc.tile_pool(name="pso", bufs=4, space="PSUM") as pso, \
         tc.tile_pool(name="dscr", bufs=4, space="DRAM") as dscr:

        # K^T rows r=64h+d live at (partition r%128, subtile r//128)
        k_sb = per.tile([P, 4, T], F32)
        # V: [t2 partition, t-tile, head, 65]; cols 0-63 = V, col 64 = ones
        v_sb = per.tile([P, NT, HL, 65], F32)
        oT_sb = per.tile([P, 4, T], F32)
        mask_sb = per.tile([P, P], F32)
        wv_sb = per.tile([P, KS, CW], F32, tag="wbig")

        nc.sync.dma_start(mask_sb, msk)
        nc.sync.dma_start(wv_sb, wv)
        nc.vector.memset(v_sb[:, :, :, 64:65], 1.0)

        for half in range(2):
            t0 = half * 1024
            xt = strm.tile([P, KS, 1024], F32, tag="xt", bufs=1)
            for ks in range(KS):
                nc.sync.dma_start(xt[:, ks, :], xT[:, ks, t0:t0 + 1024])
            q_sb = strm.tile([P, 4, 1024], F32, tag="q", bufs=1)

            # ---- phase A: Q^T (mt 0-3) and K^T (mt 4-7) for this half ----
            for mt in (4, 0, 5, 1, 6, 2, 7, 3):
                w_t = strm.tile([P, KS, P], F32, tag="wqk")
                nc.sync.dma_start(w_t, wqk[mt])
                ps_a = pss.tile([P, 1024], F32, tag="s")
                for cc in range(2):
                    for ks in range(KS):
                        nc.tensor.matmul(
                            ps_a[:, cc * 512:(cc + 1) * 512],
                            lhsT=w_t[:, ks, :],
                            rhs=xt[:, ks, cc * 512:(cc + 1) * 512],
                            start=(ks == 0), stop=(ks == KS - 1))
                if mt < 4:
                    nc.vector.tensor_copy(out=q_sb[:, mt, :], in_=ps_a)
                else:
                    nc.vector.tensor_copy(out=k_sb[:, mt - 4, t0:t0 + 1024],
                                          in_=ps_a)

            # ---- phase B: V for this half's 8 token tiles ----
            for tt8 in range(8):
                tt = half * 8 + tt8
                ps_vf = pso.tile([P, 1024], F32, tag="o2", bufs=2,
                                 name="ps_vf")
                ps_v = ps_vf[:, 0:CW]
                for ks in range(KS):
                    nc.tensor.matmul(
                        ps_v,
                        lhsT=xt[:, ks, tt8 * 128:(tt8 + 1) * 128],
                        rhs=wv_sb[:, ks, :],
                        start=(ks == 0), stop=(ks == KS - 1))
                nc.vector.tensor_copy(out=v_sb[:, tt, :, 0:64],
                                      in_=ps_v.rearrange("p (h d) -> p h d", h=HL))

            # ---- phase C: attention for this half's two q-chunks ----
            for cc in range(2):
                c = half * 2 + cc
                ntile = 4 * c + 4
                for pr in range(4):
                    h0, h1 = 2 * pr, 2 * pr + 1
                    o01 = pso.tile([P, 1024], F32, tag="o2", bufs=2)
                    o0 = o01[:, 0:512]
                    o1 = o01[:, 512:1024]
                    # 2-deep software pipeline: emit S/mask/exp for tile
                    # tt, and the P@V for tile tt-2, so trailing P@Vs run on
                    # already-exp'd tiles while the last exps overlap them.
                    DEPTH = 2
                    p_ts = {}

                    def emit_pv(tt):
                        i = tt - 4 * c
                        col0 = 128 * i if i >= 0 else 0
                        st, sp = (tt == 0), (tt == ntile - 1)
                        p_t = p_ts.pop(tt)
                        nc.tensor.matmul(
                            o0[0:65, col0:512],
                            lhsT=v_sb[:, tt, h0, 0:65],
                            rhs=p_t[:, 0, col0:512], start=st, stop=sp,
                            skip_group_check=True)
                        nc.tensor.matmul(
                            o1[0:65, col0:512],
                            lhsT=v_sb[:, tt, h1, 0:65],
                            rhs=p_t[:, 1, col0:512], start=st, stop=sp,
                            skip_group_check=True)

                    for tt in range(ntile):
                        i = tt - 4 * c  # diagonal index (>=0 on diagonal)
                        col0 = 128 * i if i >= 0 else 0
                        s_ps = pss.tile([P, 2, 512], F32, tag="s")
                        for hh, pb in ((0, 0), (1, 64)):
                            nc.tensor.matmul(
                                s_ps[:, hh, col0:512],
                                lhsT=k_sb[pb:pb + 64, pr, tt * 128:(tt + 1) * 128],
                                rhs=q_sb[pb:pb + 64, pr,
                                         cc * 512 + col0:cc * 512 + 512],
                                start=True, stop=True)
                        if i >= 0:
                            nc.vector.tensor_tensor(
                                out=s_ps[:, :, col0:col0 + 128],
                                in0=s_ps[:, :, col0:col0 + 128],
                                in1=mask_sb[:, None, :].to_broadcast((P, 2, P)),
                                op=Add)
                        p_t = pp.tile([P, 2, 512], F32, tag="p", bufs=4)
                        p_ts[tt] = p_t
                        nc.scalar.activation(
                            p_t[:, :, col0:512], s_ps[:, :, col0:512],
                            Exp, scale=SCALE)
                        if tt >= DEPTH:
                            emit_pv(tt - DEPTH)
                    for tt in range(max(0, ntile - DEPTH), ntile):
                        emit_pv(tt)
                    # normalize: O^T[h] = O'^T[h] * (1/denom[h]).  The
                    # reciprocal of the denominator row is replicated to 64
                    # partitions by a DRAM-roundtrip broadcast DMA (stride-0
                    # DRAM source APs are legal; SBUF ones are not), then
                    # multiplied in.  Odd heads get a partition-shifting
                    # SBUF->SBUF DMA into the upper half of O^T.
                    cs = slice(c * 512, (c + 1) * 512)
                    c01 = pp.tile([P, 1024], F32, tag="r", bufs=4)
                    b0 = pp.tile([P, 1024], F32, tag="r", bufs=4)
                    t1s = pp.tile([P, 512], F32, tag="r2", bufs=2)
                    scr0 = dscr.tile([1, 1024], F32)
                    # single copy frees the psum bank for the next pair's PV;
                    # reciprocal of both denominator rows lands in the copy
                    nc.vector.tensor_copy(out=c01[0:64, :], in_=o01[0:64, :])
                    nc.vector.reciprocal(c01[64:65, :], o01[64:65, :])
                    nc.sync.dma_start(scr0, c01[64:65, :])
                    nc.sync.dma_start(b0[0:64, :],
                                      scr0.to_broadcast((64, 1024)))
                    nc.vector.tensor_mul(oT_sb[0:64, pr, cs],
                                         c01[0:64, 0:512], b0[0:64, 0:512])
                    nc.vector.tensor_mul(t1s[0:64, :], c01[0:64, 512:1024],
                                         b0[0:64, 512:1024])
                    nc.sync.dma_start(oT_sb[64:128, pr, cs], t1s[0:64, :])

        # ---- phase D: partial output projection ----
        wp_sb = per.tile([P, 4, C], F32, tag="wbig")
        nc.sync.dma_start(wp_sb, wp)
        for mt in range(NT):
            ps_y = pss.tile([P, 1024], F32, tag="s")
            for jo in range(4):
                for nn in range(2):
                    nc.tensor.matmul(
                        ps_y[:, nn * 512:(nn + 1) * 512],
                        lhsT=oT_sb[:, jo, mt * 128:(mt + 1) * 128],
                        rhs=wp_sb[:, jo, nn * 512:(nn + 1) * 512],
                        start=(jo == 0), stop=(jo == 3))
            y_sb = pp.tile([P, C], F32, tag="p", bufs=4)
            nc.scalar.copy(y_sb, ps_y)
            nc.sync.dma_start(y[mt], y_sb)

    nc.compile()
    return nc


def get_module():
    if "nc" not in _CACHE:
        _CACHE["nc"] = _build_module()
    return _CACHE["nc"]


def _wp_perm():
    # O^T row layout: (partition p, subtile jo) <-> head h = 2*jo + (p>=64),
    # dim d = p % 64; w_proj row (within this core's 512) = 64*h + d.
    p = np.arange(P)[:, None]
    jo = np.arange(4)[None, :]
    h = 2 * jo + (p >= 64)
    return (64 * h + p % 64).reshape(-1)


def make_core_inputs(x, w_qkv, w_proj, core):
    b, g = core // TP, core % TP
    xt = np.ascontiguousarray(x[b].T)                    # [C, T]
    xt = np.ascontiguousarray(xt.reshape(KS, P, T).transpose(1, 0, 2))
    qcols = w_qkv[:, g * CW:(g + 1) * CW]
    kcols = w_qkv[:, C + g * CW:C + (g + 1) * CW]
    wqk = np.concatenate([qcols, kcols], axis=1)         # [C, 1024]
    wqk = np.ascontiguousarray(
        wqk.reshape(KS, P, 8, P).transpose(2, 1, 0, 3))  # [mt, p, ko, m]
    wv = w_qkv[:, 2 * C + g * CW:2 * C + (g + 1) * CW]
    wv = np.ascontiguousarray(wv.reshape(KS, P, CW).transpose(1, 0, 2))
    wp = np.ascontiguousarray(
        w_proj[g * CW:(g + 1) * CW, :][_wp_perm()].reshape(P, 4, C))
    mask = np.where(np.arange(P)[:, None] <= np.arange(P)[None, :],
                    np.float32(0.0), np.float32(MASK_NEG))
    return {"xT": xt, "wqk": wqk, "wv": wv, "wp": wp,
            "msk": np.ascontiguousarray(mask, np.float32)}


def _run(inputs, trace=False):
    x = np.asarray(inputs["x"], np.float32)
    w_qkv = np.asarray(inputs["w_qkv"], np.float32)
    w_proj = np.asarray(inputs["w_proj"], np.float32)
    b_proj = np.asarray(inputs["b_proj"], np.float32)
    nc = get_module()
    in_maps = [make_core_inputs(x, w_qkv, w_proj, core)
               for core in range(NCORES)]
    res = run_bass_kernel_spmd(nc, in_maps, core_ids=list(range(NCORES)),
                               trace=trace)
    outs = [np.asarray(r["y"], np.float32).reshape(T, C) for r in res.results]
    yfull = np.empty((B, T, C), np.float32)
    for b in range(B):
        yfull[b] = outs[TP * b] + outs[TP * b + 1] + b_proj[None, :]
    return yfull, res


def kernel(**inputs):
    y, _ = _run(inputs, trace=False)
    return y



# revision 2
# speedup vs baseline: 3.6909x; 3.6909x over previous
"""Causal self-attention (B=4, T=2048, C=1024, H=16) on 8 trn2 NeuronCores.

Sharding: data-parallel over batch (4) x tensor-parallel over heads (2 groups
of 8).  Core c handles batch c//2, head group c%2.  The host sums the two
TP partials per batch and adds the bias.

v8 = v7 (all-bf16 matmuls, P^T@[V|1] attention with per-partition
denominators, PE-transpose back to O^T, ACT=exp-only engine split) with
micro-interleaving: the attention inner loop is exp(ACT)-paced (~1us/tile)
while its own S+PV matmuls only fill ~0.6us, and PE executes in program
order -- so the PE-dense work (QKV/V/proj units) is chopped into single-
matmul steps held in a global generator queue, and each tt iteration pumps
just enough steps (budgeted in estimated PE-ns per chunk) to fill that
tile's deficit.  oN is per-chunk so proj units never false-share with the
running chunk's normalize.  Startup DMAs are ordered so the first QKV
matmul's operands land first.
"""

import sys

sys.path.insert(0, "/opt/trn_rl_repo")

import numpy as np
import ml_dtypes

import concourse.bass as bass
import concourse.bacc as bacc
import concourse.mybir as mybir
import concourse.tile as tile
from concourse.bass_utils import run_bass_kernel_spmd

F32 = mybir.dt.float32
BF = mybir.dt.bfloat16
NPBF = ml_dtypes.bfloat16
P = 128
B, T, C = 4, 2048, 1024
H, D = 16, 64
NCORES = 8
TP = 2               # head-parallel groups
HL = H // TP         # 8 heads per core
CW = HL * D          # 512 head-cols per core
KS = C // P          # 8 contraction subtiles
NT = T // P          # 16 token tiles
SCALE = float(1.0 / np.sqrt(D))

_CACHE = {}


def _build_module():
    nc = bacc.Bacc("TRN2", target_bir_lowering=False, debug=False,
                   num_devices=NCORES)
    xT = nc.dram_tensor("xT", (P, KS, T), BF, kind="ExternalInput").ap()
    wqk = nc.dram_tensor("wqk", (8, P, KS, P), BF, kind="ExternalInput").ap()
    wv = nc.dram_tensor("wv", (P, KS, CW), BF, kind="ExternalInput").ap()
    wp = nc.dram_tensor("wp", (P, 4, C), BF, kind="ExternalInput").ap()
    msk = nc.dram_tensor("msk", (P, P), BF, kind="ExternalInput").ap()
    idn = nc.dram_tensor("idn", (P, P), BF, kind="ExternalInput").ap()
    y = nc.dram_tensor("y", (NT, P, C), F32, kind="ExternalOutput").ap()

    Exp = mybir.ActivationFunctionType.Exp

    with tile.TileContext(nc) as tc, \
         tc.tile_pool(name="per", bufs=1) as per, \
         tc.tile_pool(name="strm", bufs=2) as strm, \
         tc.tile_pool(name="pp", bufs=2) as pp, \
         tc.tile_pool(name="pss", bufs=2, space="PSUM") as pss, \
         tc.tile_pool(name="pso", bufs=1, space="PSUM") as pso, \
         tc.tile_pool(name="pab", bufs=2, space="PSUM") as pab:

        # K^T rows r=64h+d live at (partition r%128, subtile r//128)
        k_sb = per.tile([P, 4, T], BF)
        # V: [t2 partition, t-tile, head, 65]; cols 0-63 = V, col 64 = ones
        v_sb = per.tile([P, NT, HL, 65], BF)
        mask_sb = per.tile([P, P], BF)
        idn_sb = per.tile([P, P], BF)
        wv_sb = per.tile([P, KS, CW], BF)
        wp_sb = per.tile([P, 4, C], BF)

        nc.scalar.dma_start(mask_sb, msk)
        nc.scalar.dma_start(idn_sb, idn)
        nc.vector.memset(v_sb[:, :, :, 64:65], 1.0)

        q_sbs, xts = {}, {}
        # normalized attention output per chunk, [q-part, tile-in-chunk,
        # head, d]; separate tiles so proj reads never wait on a newer
        # chunk's normalize writes.
        oN = {}

        def get_oN(c):
            if c not in oN:
                oN[c] = pp.tile([P, 4, HL, D], BF, tag="on", bufs=4,
                                name="oN")
            return oN[c]

        MM = 213   # est. PE ns of one 512-row bf16 matmul step

        def a_dma(mt, eng):
            w_t = strm.tile([P, KS, P], BF, tag="wqk", bufs=8, name="w_t")
            eng.dma_start(w_t, wqk[mt])
            return w_t

        def a_chain(half, mt, w_t, cc):
            """One 512-token half of a Q^T/K^T feature block."""
            t0 = half * 1024
            xt, q_sb = xts[half], q_sbs[half]
            ps_a = pab.tile([P, 512], F32, tag="ab", name="ps_a")
            for ks in range(KS):
                nc.tensor.matmul(
                    ps_a,
                    lhsT=w_t[:, ks, :],
                    rhs=xt[:, ks, cc * 512:(cc + 1) * 512],
                    start=(ks == 0), stop=(ks == KS - 1))
                yield MM
            if mt < 4:
                nc.vector.tensor_copy(
                    out=q_sb[:, mt, cc * 512:(cc + 1) * 512], in_=ps_a)
            else:
                nc.vector.tensor_copy(
                    out=k_sb[:, mt - 4,
                             t0 + cc * 512:t0 + (cc + 1) * 512],
                    in_=ps_a)
            yield 0

        def a_steps(half, mt):
            """Filler unit over prefetched weights."""
            for cc in range(2):
                for est in a_chain(half, mt, wts1[mt], cc):
                    yield est

        def b_steps(half, tt8):
            """V (all 8 heads + implicit ones col) for one token tile."""
            xt = xts[half]
            tt = half * 8 + tt8
            ps_v = pab.tile([P, 512], F32, tag="ab", name="ps_v")
            for ks in range(KS):
                nc.tensor.matmul(
                    ps_v,
                    lhsT=xt[:, ks, tt8 * 128:(tt8 + 1) * 128],
                    rhs=wv_sb[:, ks, :],
                    start=(ks == 0), stop=(ks == KS - 1))
                yield MM
            nc.vector.tensor_copy(
                out=v_sb[:, tt, :, 0:64],
                in_=ps_v.rearrange("p (h d) -> p h d", h=HL))
            yield 0

        def d_steps(mt, tail=False):
            """O^T transpose + output projection + store for one token
            tile.  Tail units put the oT copy on ACT (idle after C3)."""
            oNc = get_oN(mt // 4)
            mtl = mt % 4
            tr = pab.tile([P, 4, P], BF, tag="ab", name="tr")
            for jo in range(4):
                nc.tensor.transpose(tr[:, jo, :],
                                    oNc[:, mtl, 2 * jo:2 * jo + 2, :],
                                    idn_sb)
                yield 53
            oTt = pp.tile([P, 4, P], BF, tag="ot", bufs=2)
            if tail:
                nc.scalar.copy(oTt, tr)
            else:
                nc.vector.tensor_copy(out=oTt, in_=tr)
            yield 0
            ps_y = pss.tile([P, 1024], F32, tag="s")
            for jo in range(4):
                for nn in range(2):
                    nc.tensor.matmul(
                        ps_y[:, nn * 512:(nn + 1) * 512],
                        lhsT=oTt[:, jo, :],
                        rhs=wp_sb[:, jo, nn * 512:(nn + 1) * 512],
                        start=(jo == 0), stop=(jo == 3))
                    yield MM
            y_sb = pp.tile([P, C], F32, tag="y", bufs=2)
            nc.vector.tensor_copy(out=y_sb, in_=ps_y)
            yield 0
            (nc.sync if tail else nc.scalar).dma_start(y[mt], y_sb)
            yield 0

        micro = []

        def pump(budget):
            while budget > 0 and micro:
                try:
                    budget -= next(micro[0])
                except StopIteration:
                    micro.pop(0)

        def flush():
            while micro:
                pump(1 << 30)

        def phase_C(c):
            """Attention for chunk c; pumps filler steps to cover the gap
            between exp (ACT) pace and this chunk's own S+PV PE work."""
            half, cc = c // 2, c % 2
            ntile = 4 * c + 4
            q_sb = q_sbs[half]
            oNc = get_oN(c)
            carry = 0.0
            for pr in range(4):
                h0 = 2 * pr
                # o accumulators: bank b, slot jj=(qs&1)*2+hh at cols
                # jj*65..jj*65+65; qs = 2*b + (jj>>1).  Column 64 of each
                # slot accumulates the softmax denominator (ones column).
                o_ps = pso.tile([P, 2, 512], F32, tag="o2")
                p_ts = {}
                DEPTH = 2

                def emit_pv(tt):
                    i = tt - 4 * c
                    p_t = p_ts.pop(tt)
                    st = (tt == 0)
                    npv = 0
                    for qs in range(max(i, 0), 4):
                        qt = 4 * c + qs
                        b, j0 = qs >> 1, (qs & 1) * 2
                        for hh in range(2):
                            nc.tensor.matmul(
                                o_ps[:, b, (j0 + hh) * 65:(j0 + hh + 1) * 65],
                                lhsT=p_t[:, hh, qs * 128:(qs + 1) * 128],
                                rhs=v_sb[:, tt, h0 + hh, 0:65],
                                start=st, stop=(tt == qt),
                                skip_group_check=True)
                            npv += 1
                    if i >= 0:
                        # slot qs=i's chains just stopped: normalize now so
                        # o_ps streams free instead of lumping at pr end
                        qs = i
                        b, j0 = qs >> 1, (qs & 1) * 2
                        sl = o_ps[:, b, j0 * 65:j0 * 65 + 130].rearrange(
                            "p (h e) -> p h e", e=65)
                        rec = pp.tile([P, 2], F32, tag="r", bufs=4,
                                      name="rec")
                        nc.vector.reciprocal(rec[:, :, None], sl[:, :, 64:65])
                        nc.vector.tensor_mul(
                            oNc[:, qs, h0:h0 + 2, :],
                            sl[:, :, 0:64],
                            rec[:, :, None].to_broadcast((P, 2, 64)))
                    return npv

                for tt in range(ntile):
                    i = tt - 4 * c  # diagonal index (>=0 on diagonal)
                    col0 = 128 * i if i >= 0 else 0
                    s_ps = pss.tile([P, 2, 512], F32, tag="s")
                    for hh, pb in ((0, 0), (1, 64)):
                        nc.tensor.matmul(
                            s_ps[:, hh, col0:512],
                            lhsT=k_sb[pb:pb + 64, pr, tt * 128:(tt + 1) * 128],
                            rhs=q_sb[pb:pb + 64, pr,
                                     cc * 512 + col0:cc * 512 + 512],
                            start=True, stop=True)
                    p_t = pp.tile([P, 2, 512], BF, tag="p", bufs=5)
                    p_ts[tt] = p_t
                    nc.scalar.activation(
                        p_t[:, :, col0:512], s_ps[:, :, col0:512],
                        Exp, scale=SCALE)
                    if i >= 0:
                        # multiplicative 0/1 mask on the diagonal slab
                        nc.gpsimd.tensor_mul(
                            p_t[:, :, col0:col0 + 128],
                            p_t[:, :, col0:col0 + 128],
                            mask_sb[:, None, :].to_broadcast((P, 2, P)))
                    npv = emit_pv(tt - DEPTH) if tt >= DEPTH else 0
                    # exp pace minus this iteration's own S+PV work
                    rows = 2 * (512 - col0)
                    deficit = (rows * 0.833 + 185) \
                        - (rows * 0.4167 + npv * 65 * 0.4167)
                    if tt < ntile - 2:
                        pump(deficit)
                    else:
                        carry += deficit
                for tt in range(max(0, ntile - DEPTH), ntile):
                    emit_pv(tt)
                pump(carry + 500)
                carry = 0.0

        # -------- emission schedule --------
        for half in range(2):
            xts[half] = strm.tile([P, KS, 1024], BF, tag="xt", bufs=2,
                                  name="xt")
            q_sbs[half] = strm.tile([P, 4, 1024], BF, tag="q", bufs=2,
                                    name="q_sb")

        # startup: the first chain's operands land first.  SP carries the
        # critical path (wt4, x column-halves); ACT (behind its act-table
        # load) carries the rest of the weights.
        A0 = (4, 0, 5, 1, 6, 2, 7, 3)
        wts = {}
        wts[4] = a_dma(4, nc.sync)
        nc.sync.dma_start(xts[0][:, 0:2, 0:512], xT[:, 0:2, 0:512])
        wts[0] = a_dma(0, nc.sync)
        for ks in range(2, KS, 2):
            nc.sync.dma_start(xts[0][:, ks:ks + 2, 0:512],
                              xT[:, ks:ks + 2, 0:512])
        for mt in (5, 1, 6, 2, 7, 3):
            wts[mt] = a_dma(mt, nc.scalar)
        nc.sync.dma_start(xts[0][:, :, 512:1024], xT[:, :, 512:1024])
        nc.sync.dma_start(wv_sb, wv)
        for cc in range(2):
            for mt in A0:
                for _ in a_chain(0, mt, wts[mt], cc):
                    pass
        nc.sync.dma_start(xts[1][:, :, 0:512], xT[:, :, 1024:1536])
        nc.sync.dma_start(xts[1][:, :, 512:1024], xT[:, :, 1536:2048])
        nc.sync.dma_start(wp_sb, wp)
        for tt8 in range(8):
            for _ in b_steps(0, tt8):
                pass
        wts1 = {mt: a_dma(mt, nc.sync) for mt in (4, 0, 5, 1, 6, 2, 7, 3)}

        micro.extend([b_steps(1, 0), b_steps(1, 1)])
        micro.extend([a_steps(1, m) for m in (4, 0, 5, 1)])
        micro.extend([b_steps(1, 2), b_steps(1, 3)])
        micro.extend([a_steps(1, m) for m in (6, 2, 7, 3)])
        micro.extend([b_steps(1, t) for t in range(4, 8)])
        micro.extend([d_steps(m) for m in range(12)])

        phase_C(0)
        phase_C(1)
        phase_C(2)
        phase_C(3)
        flush()
        for mt in range(12, 16):
            for _ in d_steps(mt, tail=True):
                pass

    nc.compile()
    return nc


def get_module():
    if "nc" not in _CACHE:
        _CACHE["nc"] = _build_module()
    return _CACHE["nc"]


def _wp_perm():
    # O^T row layout: (partition p, subtile jo) <-> head h = 2*jo + (p>=64),
    # dim d = p % 64; w_proj row (within this core's 512) = 64*h + d.
    p = np.arange(P)[:, None]
    jo = np.arange(4)[None, :]
    h = 2 * jo + (p >= 64)
    return (64 * h + p % 64).reshape(-1)


def make_core_inputs(x, w_qkv, w_proj, core):
    b, g = core // TP, core % TP
    xt = np.ascontiguousarray(x[b].T)                    # [C, T]
    xt = np.ascontiguousarray(xt.reshape(KS, P, T).transpose(1, 0, 2))
    qcols = w_qkv[:, g * CW:(g + 1) * CW]
    kcols = w_qkv[:, C + g * CW:C + (g + 1) * CW]
    wqk = np.concatenate([qcols, kcols], axis=1)         # [C, 1024]
    wqk = np.ascontiguousarray(
        wqk.reshape(KS, P, 8, P).transpose(2, 1, 0, 3))  # [mt, p, ko, m]
    wv = w_qkv[:, 2 * C + g * CW:2 * C + (g + 1) * CW]
    wv = np.ascontiguousarray(wv.reshape(KS, P, CW).transpose(1, 0, 2))
    wpp = np.ascontiguousarray(
        w_proj[g * CW:(g + 1) * CW, :][_wp_perm()].reshape(P, 4, C))
    mask = np.where(np.arange(P)[:, None] <= np.arange(P)[None, :],
                    np.float32(1.0), np.float32(0.0))
    return {"xT": xt.astype(NPBF), "wqk": wqk.astype(NPBF),
            "wv": wv.astype(NPBF), "wp": wpp.astype(NPBF),
            "msk": mask.astype(NPBF),
            "idn": np.eye(P, dtype=np.float32).astype(NPBF)}


def _run(inputs, trace=False):
    x = np.asarray(inputs["x"], np.float32)
    w_qkv = np.asarray(inputs["w_qkv"], np.float32)
    w_proj = np.asarray(inputs["w_proj"], np.float32)
    b_proj = np.asarray(inputs["b_proj"], np.float32)
    nc = get_module()
    in_maps = [make_core_inputs(x, w_qkv, w_proj, core)
               for core in range(NCORES)]
    res = run_bass_kernel_spmd(nc, in_maps, core_ids=list(range(NCORES)),
                               trace=trace)
    outs = [np.asarray(r["y"], np.float32).reshape(T, C) for r in res.results]
    yfull = np.empty((B, T, C), np.float32)
    for b in range(B):
        yfull[b] = outs[TP * b] + outs[TP * b + 1] + b_proj[None, :]
    return yfull, res


def kernel(**inputs):
    y, _ = _run(inputs, trace=False)
    return y


# revision 3
# speedup vs baseline: 3.7798x; 1.0241x over previous
"""Causal self-attention (B=4, T=2048, C=1024, H=16) on 8 trn2 NeuronCores.

Sharding: data-parallel over batch (4) x tensor-parallel over heads (2 groups
of 8).  Core c handles batch c//2, head group c%2.  The host sums the two
TP partials per batch and adds the bias.

Implementation notes (vs the fp32 v1 baseline, 935us -> ~247us):
  - all matmul operands in bf16 (fp32 PSUM accumulation): PE runs at
    1 cycle/row instead of fp32's 4.  rel err ~5e-3 << the 2e-2 gate.
  - P@V runs as P^T@[V|1]: out [128 queries, 65] per (head, q-subtile), so
    the PE moves 65 rows per 128x128 key tile instead of 512-col0, and the
    softmax denominator lands in column 64 as a per-partition scalar
    (normalize = DVE reciprocal + broadcast multiply, streamed per slot as
    each accumulation chain stops).  PSUM zero regions are 2KB: only the
    FIRST slot in each psum bank issues start_tensor_calc; sibling slots
    rely on the region's pending-zero write-through for their reset.
  - O ([q,d] layout) returns to O^T via PE transposes against a bf16
    identity (single start/stop per psum bank), so the output projection
    consumes O^T as lhsT; host pre-permutes w_proj rows to match.
  - engines: ACT = exp only; DVE = psum->sbuf copies + normalize; GpSimd =
    multiplicative 0/1 diagonal mask (post-exp, bf16); y stores in bf16
    (host upcasts + sums the TP partials).
  - the attention inner loop is exp(ACT)-paced (~1us/tile) while its own
    S+PV matmuls fill only ~0.6us, and engines execute in program order --
    so ALL PE-dense work (QKV q/k/v projection units for both token
    halves beyond the first chunk's inputs, and transpose+proj+store units
    per finished chunk) is chopped into single-matmul generator steps in a
    deadline-ordered queue; each tt iteration pumps just enough steps
    (estimated PE-ns) to fill that tile's deficit, and require() force-
    drains the queue so a consumer is never emitted before its producer.
  - oN is per-chunk so proj units never false-share with the running
    chunk's normalize; startup DMAs are ordered so the first QKV matmul's
    operands land first (weights on SP ahead of x column-halves; the ACT
    hwdge queue carries the remaining weights behind its act-table load).
"""

import sys

sys.path.insert(0, "/opt/trn_rl_repo")

import numpy as np
import ml_dtypes

import concourse.bass as bass
import concourse.bacc as bacc
import concourse.mybir as mybir
import concourse.tile as tile
from concourse.bass_utils import run_bass_kernel_spmd

F32 = mybir.dt.float32
BF = mybir.dt.bfloat16
NPBF = ml_dtypes.bfloat16
P = 128
B, T, C = 4, 2048, 1024
H, D = 16, 64
NCORES = 8
TP = 2               # head-parallel groups
HL = H // TP         # 8 heads per core
CW = HL * D          # 512 head-cols per core
KS = C // P          # 8 contraction subtiles
NT = T // P          # 16 token tiles
SCALE = float(1.0 / np.sqrt(D))

_CACHE = {}


def _build_module():
    nc = bacc.Bacc("TRN2", target_bir_lowering=False, debug=False,
                   num_devices=NCORES)
    xT = nc.dram_tensor("xT", (P, KS, T), BF, kind="ExternalInput").ap()
    wqk = nc.dram_tensor("wqk", (8, P, KS, P), BF, kind="ExternalInput").ap()
    wv = nc.dram_tensor("wv", (P, KS, CW), BF, kind="ExternalInput").ap()
    wp = nc.dram_tensor("wp", (P, 4, C), BF, kind="ExternalInput").ap()
    msk = nc.dram_tensor("msk", (P, P), BF, kind="ExternalInput").ap()
    idn = nc.dram_tensor("idn", (P, P), BF, kind="ExternalInput").ap()
    y = nc.dram_tensor("y", (NT, P, C), BF, kind="ExternalOutput").ap()

    Exp = mybir.ActivationFunctionType.Exp

    with tile.TileContext(nc) as tc, \
         tc.tile_pool(name="per", bufs=1) as per, \
         tc.tile_pool(name="strm", bufs=2) as strm, \
         tc.tile_pool(name="pp", bufs=2) as pp, \
         tc.tile_pool(name="pss", bufs=2, space="PSUM") as pss, \
         tc.tile_pool(name="pso", bufs=1, space="PSUM") as pso, \
         tc.tile_pool(name="pab", bufs=2, space="PSUM") as pab:

        # K^T rows r=64h+d live at (partition r%128, subtile r//128)
        k_sb = per.tile([P, 4, T], BF)
        # V: [t2 partition, t-tile, head, 65]; cols 0-63 = V, col 64 = ones
        v_sb = per.tile([P, NT, HL, 65], BF)
        mask_sb = per.tile([P, P], BF)
        idn_sb = per.tile([P, P], BF)
        wv_sb = per.tile([P, KS, CW], BF)
        wp_sb = per.tile([P, 4, C], BF)

        nc.vector.memset(v_sb[:, :, :, 64:65], 1.0)

        q_sbs, xts = {}, {}
        # normalized attention output per chunk, [q-part, tile-in-chunk,
        # head, d]; separate tiles so proj reads never wait on a newer
        # chunk's normalize writes.
        oN = {}

        def get_oN(c):
            if c not in oN:
                oN[c] = pp.tile([P, 4, HL, D], BF, tag="on", bufs=4,
                                name="oN")
            return oN[c]

        MM = 213   # est. PE ns of one 512-row bf16 matmul step

        def a_dma(mt, eng):
            w_t = strm.tile([P, KS, P], BF, tag="wqk", bufs=16, name="w_t")
            eng.dma_start(w_t, wqk[mt])
            return w_t

        def a_chain(half, mt, w_t, cc):
            """One 512-token half of a Q^T/K^T feature block."""
            t0 = half * 1024
            xt, q_sb = xts[half], q_sbs[half]
            ps_a = pab.tile([P, 512], F32, tag="ab", name="ps_a")
            for ks in range(KS):
                nc.tensor.matmul(
                    ps_a,
                    lhsT=w_t[:, ks, :],
                    rhs=xt[:, ks, cc * 512:(cc + 1) * 512],
                    start=(ks == 0), stop=(ks == KS - 1))
                yield MM
            if mt < 4:
                nc.vector.tensor_copy(
                    out=q_sb[:, mt, cc * 512:(cc + 1) * 512], in_=ps_a)
            else:
                nc.vector.tensor_copy(
                    out=k_sb[:, mt - 4,
                             t0 + cc * 512:t0 + (cc + 1) * 512],
                    in_=ps_a)
            yield 0

        def a_steps(half, mt):
            """Filler unit over prefetched weights."""
            for cc in range(2):
                for est in a_chain(half, mt, wts1[mt], cc):
                    yield est

        def b_steps(half, tt8):
            """V (all 8 heads + implicit ones col) for one token tile."""
            xt = xts[half]
            tt = half * 8 + tt8
            ps_v = pab.tile([P, 512], F32, tag="ab", name="ps_v")
            for ks in range(KS):
                nc.tensor.matmul(
                    ps_v,
                    lhsT=xt[:, ks, tt8 * 128:(tt8 + 1) * 128],
                    rhs=wv_sb[:, ks, :],
                    start=(ks == 0), stop=(ks == KS - 1))
                yield MM
            nc.vector.tensor_copy(
                out=v_sb[:, tt, :, 0:64],
                in_=ps_v.rearrange("p (h d) -> p h d", h=HL))
            yield 0

        def d_steps(mt, tail=False):
            """O^T transpose + output projection + store for one token
            tile.  Tail units put the oT copy on ACT (idle after C3)."""
            oNc = get_oN(mt // 4)
            mtl = mt % 4
            tr = pab.tile([P, 4, P], BF, tag="ab", name="tr")
            for jo in range(4):
                nc.tensor.matmul(tr[:, jo, :],
                                 lhsT=oNc[:, mtl, 2 * jo:2 * jo + 2, :],
                                 rhs=idn_sb, is_transpose=True,
                                 start=(jo == 0), stop=(jo == 3),
                                 skip_group_check=True)
                yield 53
            oTt = pp.tile([P, 4, P], BF, tag="ot", bufs=2)
            if tail:
                nc.scalar.copy(oTt, tr)
            else:
                nc.vector.tensor_copy(out=oTt, in_=tr)
            yield 0
            ps_y = pss.tile([P, 1024], F32, tag="s")
            for jo in range(4):
                for nn in range(2):
                    nc.tensor.matmul(
                        ps_y[:, nn * 512:(nn + 1) * 512],
                        lhsT=oTt[:, jo, :],
                        rhs=wp_sb[:, jo, nn * 512:(nn + 1) * 512],
                        start=(jo == 0), stop=(jo == 3))
                    yield MM
            y_sb = pp.tile([P, C], BF, tag="y", bufs=2)
            if tail:
                for nn in range(2):
                    nc.vector.tensor_copy(
                        out=y_sb[:, nn * 512:(nn + 1) * 512],
                        in_=ps_y[:, nn * 512:(nn + 1) * 512])
                    yield 0
                    nc.sync.dma_start(y[mt][:, nn * 512:(nn + 1) * 512],
                                      y_sb[:, nn * 512:(nn + 1) * 512])
                    yield 0
            else:
                nc.vector.tensor_copy(out=y_sb, in_=ps_y)
                yield 0
                nc.scalar.dma_start(y[mt], y_sb)
                yield 0

        micro = []          # list of (name, generator)
        done = set()

        def pump(budget):
            while budget > 0 and micro:
                try:
                    budget -= next(micro[0][1])
                except StopIteration:
                    done.add(micro.pop(0)[0])

        def require(*names):
            """Force-drain the queue until the named units are emitted, so
            consumers are never emitted before their producers."""
            for name in names:
                if name in done or not any(n == name for n, _ in micro):
                    continue
                while name not in done and micro:
                    try:
                        next(micro[0][1])
                    except StopIteration:
                        done.add(micro.pop(0)[0])

        def flush():
            while micro:
                pump(1 << 30)

        def phase_C(c, bpump=0, dsc=0.95):
            """Attention for chunk c; pumps filler steps to cover the gap
            between exp (ACT) pace and this chunk's own S+PV PE work."""
            half, cc = c // 2, c % 2
            ntile = 4 * c + 4
            q_sb = q_sbs[half]
            oNc = get_oN(c)
            carry = 0.0
            for pr in range(4):
                require(("a", half, pr), ("a", half, 4 + pr))
                h0 = 2 * pr
                # o accumulators: bank b, slot jj=(qs&1)*2+hh at cols
                # jj*65..jj*65+65; qs = 2*b + (jj>>1).  Column 64 of each
                # slot accumulates the softmax denominator (ones column).
                o_ps = pso.tile([P, 2, 512], F32, tag="o2")
                p_ts = {}
                DEPTH = 3

                def emit_pv(tt):
                    i = tt - 4 * c
                    p_t = p_ts.pop(tt)
                    npv = 0
                    for qs in range(max(i, 0), 4):
                        qt = 4 * c + qs
                        b, j0 = qs >> 1, (qs & 1) * 2
                        for hh in range(2):
                            # one start per 2KB psum zero region (bank):
                            # the bank's first slot resets it; sibling
                            # slots write-through via the pending-zero.
                            nc.tensor.matmul(
                                o_ps[:, b, (j0 + hh) * 65:(j0 + hh + 1) * 65],
                                lhsT=p_t[:, hh, qs * 128:(qs + 1) * 128],
                                rhs=v_sb[:, tt, h0 + hh, 0:65],
                                start=(tt == 0 and hh == 0 and (qs & 1) == 0),
                                stop=(tt == qt),
                                skip_group_check=True)
                            npv += 1
                    if i >= 0:
                        # slot qs=i's chains just stopped: normalize now so
                        # o_ps streams free instead of lumping at pr end
                        qs = i
                        b, j0 = qs >> 1, (qs & 1) * 2
                        sl = o_ps[:, b, j0 * 65:j0 * 65 + 130].rearrange(
                            "p (h e) -> p h e", e=65)
                        rec = pp.tile([P, 2], F32, tag="r", bufs=4,
                                      name="rec")
                        nc.vector.reciprocal(rec[:, :, None], sl[:, :, 64:65])
                        nc.vector.tensor_mul(
                            oNc[:, qs, h0:h0 + 2, :],
                            sl[:, :, 0:64],
                            rec[:, :, None].to_broadcast((P, 2, 64)))
                    return npv

                for tt in range(ntile):
                    i = tt - 4 * c  # diagonal index (>=0 on diagonal)
                    col0 = 128 * i if i >= 0 else 0
                    s_ps = pss.tile([P, 2, 512], F32, tag="s")
                    for hh, pb in ((0, 0), (1, 64)):
                        nc.tensor.matmul(
                            s_ps[:, hh, col0:512],
                            lhsT=k_sb[pb:pb + 64, pr, tt * 128:(tt + 1) * 128],
                            rhs=q_sb[pb:pb + 64, pr,
                                     cc * 512 + col0:cc * 512 + 512],
                            start=True, stop=True)
                    p_t = pp.tile([P, 2, 512], BF, tag="p", bufs=5)
                    p_ts[tt] = p_t
                    nc.scalar.activation(
                        p_t[:, :, col0:512], s_ps[:, :, col0:512],
                        Exp, scale=SCALE)
                    if i >= 0:
                        # multiplicative 0/1 mask on the diagonal slab
                        nc.gpsimd.tensor_mul(
                            p_t[:, :, col0:col0 + 128],
                            p_t[:, :, col0:col0 + 128],
                            mask_sb[:, None, :].to_broadcast((P, 2, P)))
                    if tt + 1 < ntile:
                        require(("b", tt + 1))
                    npv = emit_pv(tt - DEPTH) if tt >= DEPTH else 0
                    # exp pace minus this iteration's own S+PV work
                    rows = 2 * (512 - col0)
                    deficit = dsc * (rows * 0.833 + 185) \
                        - (rows * 0.4167 + npv * 65 * 0.4167)
                    if tt < ntile - 2:
                        pump(deficit)
                    else:
                        carry += deficit
                for tt in range(max(0, ntile - DEPTH), ntile):
                    emit_pv(tt)
                pump(carry + bpump)
                carry = 0.0

        # -------- emission schedule --------
        for half in range(2):
            xts[half] = strm.tile([P, KS, 1024], BF, tag="xt", bufs=2,
                                  name="xt")
            q_sbs[half] = strm.tile([P, 4, 1024], BF, tag="q", bufs=2,
                                    name="q_sb")

        # startup: the first chain's operands land first.  SP carries the
        # critical path (wt4, x column-halves); ACT (behind its act-table
        # load) carries the rest of the weights.
        A0 = (4, 0, 5, 1, 6, 2, 7, 3)
        wts = {}
        wts[4] = a_dma(4, nc.sync)
        nc.sync.dma_start(xts[0][:, 0:2, 0:512], xT[:, 0:2, 0:512])
        wts[0] = a_dma(0, nc.sync)
        for ks in range(2, KS, 2):
            nc.sync.dma_start(xts[0][:, ks:ks + 2, 0:512],
                              xT[:, ks:ks + 2, 0:512])
        for mt in (5, 1, 6, 2, 7, 3):
            wts[mt] = a_dma(mt, nc.scalar)
        nc.sync.dma_start(xts[0][:, :, 512:1024], xT[:, :, 512:1024])
        nc.sync.dma_start(wv_sb, wv)

        def a0_steps(mt):
            for cc in range(2):
                for est in a_chain(0, mt, wts[mt], cc):
                    yield est

        # only the first C0 block's inputs run up front (Q0/K4, V tiles
        # 0-3); the rest of QKV/V(half0) joins the filler pool
        for cc in range(2):
            for mt in (4, 0):
                for _ in a_chain(0, mt, wts[mt], cc):
                    pass
        nc.sync.dma_start(mask_sb, msk)
        nc.sync.dma_start(idn_sb, idn)
        nc.sync.dma_start(xts[1][:, :, 0:512], xT[:, :, 1024:1536])
        nc.sync.dma_start(xts[1][:, :, 512:1024], xT[:, :, 1536:2048])
        nc.sync.dma_start(wp_sb, wp)
        for tt8 in range(4):
            for _ in b_steps(0, tt8):
                pass
        wts1 = {mt: a_dma(mt, nc.sync) for mt in (4, 0, 5, 1, 6, 2, 7, 3)}

        for mt in (5, 1):
            micro.append((("a", 0, mt), a0_steps(mt)))
        for t in (4, 5):
            micro.append((("b", t), b_steps(0, t)))
        for mt in (6, 2):
            micro.append((("a", 0, mt), a0_steps(mt)))
        for t in (6, 7):
            micro.append((("b", t), b_steps(0, t)))
        for mt in (7, 3):
            micro.append((("a", 0, mt), a0_steps(mt)))
        micro.append((("a", 1, 4), a_steps(1, 4)))
        micro.append((("b", 8), b_steps(1, 0)))
        for mt in (0, 5, 1):
            micro.append((("a", 1, mt), a_steps(1, mt)))
        for t in (9, 10, 11):
            micro.append((("b", t), b_steps(1, t - 8)))
        for mt in (6, 2, 7, 3):
            micro.append((("a", 1, mt), a_steps(1, mt)))
        for t in range(12, 16):
            micro.append((("b", t), b_steps(1, t - 8)))
        for m in range(12):
            micro.append((("d", m), d_steps(m)))
        for mt in (4, 0):
            done.add(("a", 0, mt))
        for t in range(4):
            done.add(("b", t))

        phase_C(0, bpump=500)
        phase_C(1, bpump=500)
        phase_C(2, dsc=1.0)
        phase_C(3, dsc=1.0)
        flush()
        for mt in range(12, 16):
            for _ in d_steps(mt, tail=True):
                pass

    nc.compile()
    return nc


def get_module():
    if "nc" not in _CACHE:
        _CACHE["nc"] = _build_module()
    return _CACHE["nc"]


def _wp_perm():
    # O^T row layout: (partition p, subtile jo) <-> head h = 2*jo + (p>=64),
    # dim d = p % 64; w_proj row (within this core's 512) = 64*h + d.
    p = np.arange(P)[:, None]
    jo = np.arange(4)[None, :]
    h = 2 * jo + (p >= 64)
    return (64 * h + p % 64).reshape(-1)


def make_core_inputs(x, w_qkv, w_proj, core):
    b, g = core // TP, core % TP
    xt = np.ascontiguousarray(x[b].T)                    # [C, T]
    xt = np.ascontiguousarray(xt.reshape(KS, P, T).transpose(1, 0, 2))
    qcols = w_qkv[:, g * CW:(g + 1) * CW]
    kcols = w_qkv[:, C + g * CW:C + (g + 1) * CW]
    wqk = np.concatenate([qcols, kcols], axis=1)         # [C, 1024]
    wqk = np.ascontiguousarray(
        wqk.reshape(KS, P, 8, P).transpose(2, 1, 0, 3))  # [mt, p, ko, m]
    wv = w_qkv[:, 2 * C + g * CW:2 * C + (g + 1) * CW]
    wv = np.ascontiguousarray(wv.reshape(KS, P, CW).transpose(1, 0, 2))
    wpp = np.ascontiguousarray(
        w_proj[g * CW:(g + 1) * CW, :][_wp_perm()].reshape(P, 4, C))
    mask = np.where(np.arange(P)[:, None] <= np.arange(P)[None, :],
                    np.float32(1.0), np.float32(0.0))
    return {"xT": xt.astype(NPBF), "wqk": wqk.astype(NPBF),
            "wv": wv.astype(NPBF), "wp": wpp.astype(NPBF),
            "msk": mask.astype(NPBF),
            "idn": np.eye(P, dtype=np.float32).astype(NPBF)}


def _run(inputs, trace=False):
    x = np.asarray(inputs["x"], np.float32)
    w_qkv = np.asarray(inputs["w_qkv"], np.float32)
    w_proj = np.asarray(inputs["w_proj"], np.float32)
    b_proj = np.asarray(inputs["b_proj"], np.float32)
    nc = get_module()
    in_maps = [make_core_inputs(x, w_qkv, w_proj, core)
               for core in range(NCORES)]
    res = run_bass_kernel_spmd(nc, in_maps, core_ids=list(range(NCORES)),
                               trace=trace)
    outs = [np.asarray(r["y"]).astype(np.float32).reshape(T, C)
            for r in res.results]
    yfull = np.empty((B, T, C), np.float32)
    for b in range(B):
        yfull[b] = outs[TP * b] + outs[TP * b + 1] + b_proj[None, :]
    return yfull, res


def kernel(**inputs):
    y, _ = _run(inputs, trace=False)
    return y
